# revision 1
# baseline (speedup 1.0000x reference)
"""MoE FFN (nn_MoEFFN_21285857919578) — Trainium2 Bass kernel, 8 NeuronCores.

Strategy: expert-parallel. Core c owns expert c (E=8, 8 cores).
Each core: fp32 gate over all N=8192 tokens -> top-2 combine weight for its
own expert -> compaction (prefix sums via triangular matmuls) -> tiny
indirect scatter of (token+1, weight) meta pairs into xmeta[C, 2] ->
dma_gather of selected token rows -> float32r FFN h = silu((x@gpT)*(x@upT)),
y = h@dwT over compact tokens -> comb-scaled rows dma_scatter_add'ed into a
zeroed partial[N, D] -> ReduceScatter over 8 cores -> each core outputs its
N/8-token shard; host concatenates.

One SPMD program for all cores; the expert identity is carried entirely by
per-core input data (gate column 8 duplicates the own expert's gate row, so
the program never indexes by expert id).
"""
import numpy as np

import concourse.bass as bass
import concourse.tile as tile
from concourse import bacc, mybir, library_config
from concourse.bass_utils import run_bass_kernel_spmd
from concourse.masks import make_identity, make_upper_triangular
from contextlib import ExitStack

F32 = mybir.dt.float32
F32R = mybir.dt.float32r
I16 = mybir.dt.int16
I32 = mybir.dt.int32
AX = mybir.AxisListType
OP = mybir.AluOpType
ACT = mybir.ActivationFunctionType

B, S = 4, 2048
N, D, E = 8192, 1024, 8
F = 3264
FP = 3328               # F padded to 26*128 (zero-padded weights)
FB = FP // 128          # 26
KB = D // 128           # 8
NB = N // 128           # 64
E9 = E + 1
BIG = 1.0e7
NCORES = 8


def build_moe(nc, n_cores=NCORES, with_rs=True, C=2304, CHUNK=768, NMM=384,
              batched_meta_scatter=False, stage=3, dbg=False):
    assert C % CHUNK == 0 and CHUNK % NMM == 0 and CHUNK % 128 == 0
    NCH = C // CHUNK
    CB = CHUNK // 128
    N3 = CHUNK // NMM
    CW = CHUNK // 16          # idx columns per chunk (16-wrap)

    xT = nc.dram_tensor("xT", [D, N], F32, kind="ExternalInput")
    x = nc.dram_tensor("x", [N, D], F32, kind="ExternalInput")
    gwT9 = nc.dram_tensor("gwT9", [D, E9], F32, kind="ExternalInput")
    iota = nc.dram_tensor("iota", [128, NB], F32, kind="ExternalInput")
    wgT = nc.dram_tensor("wgT", [D, FP], F32R, kind="ExternalInput")
    wuT = nc.dram_tensor("wuT", [D, FP], F32R, kind="ExternalInput")
    dwT = nc.dram_tensor("dwT", [FP, D], F32R, kind="ExternalInput")

    if with_rs:
        NS = N // n_cores
        shard_o = nc.dram_tensor("shard_o", [NS, D], F32, kind="ExternalOutput")
    else:
        part_o = nc.dram_tensor("part_o", [N, D], F32, kind="ExternalOutput")
    if dbg:
        idx_o = nc.dram_tensor("idx_o", [128, C // 16], I16, kind="ExternalOutput")
        comb_o = nc.dram_tensor("comb_o", [128, C // 128], F32, kind="ExternalOutput")
        xg_o = nc.dram_tensor("xg_o", [128, CHUNK // 128, D], F32, kind="ExternalOutput")

    with tile.TileContext(nc) as tc, ExitStack() as est:
        const = est.enter_context(tc.tile_pool(name="const", bufs=1))
        rt = est.enter_context(tc.tile_pool(name="rt", bufs=1))
        dram = est.enter_context(tc.tile_pool(name="dram", bufs=1, space="DRAM"))

        nc.gpsimd.load_library(library_config.mlp)

        xmeta = dram.tile([C, 2], F32)
        partial = dram.tile([N, D], F32)

        ident = const.tile([128, 128], F32)
        make_identity(nc, ident)
        u128 = const.tile([128, 128], F32)
        make_upper_triangular(nc, u128, val=1.0, diag=False)
        ones_col = const.tile([128, 1], F32)
        nc.vector.memset(ones_col[:], 1.0)
        ones_row = const.tile([1, 128], F32)
        nc.vector.memset(ones_row[:], 1.0)
        gw_sb = const.tile([128, KB, E9], F32)
        nc.sync.dma_start(gw_sb[:], gwT9.ap().rearrange("(kb p) e -> p kb e", p=128))
        iota_sb = const.tile([128, NB], F32)
        nc.sync.dma_start(iota_sb[:], iota.ap())

        # ---- zero partial + xmeta off the gate's DMA queue ----
        zp_est = ExitStack()
        zpool = zp_est.enter_context(tc.tile_pool(name="zpool", bufs=1))
        zero_sb = zpool.tile([128, D], F32)
        nc.vector.memset(zero_sb[:], 0.0)
        nc.scalar.dma_start(
            xmeta[:].rearrange("(p c) two -> p c two", p=128),
            zero_sb[:, 0:C * 2 // 128])
        for r in range(N // 128):
            nc.scalar.dma_start(partial[r * 128:(r + 1) * 128, :], zero_sb[:])
        zp_est.close()

        # ---- gate (fp32): z.T chunks -> transpose to [128, NB, 9] ----
        zall_est = ExitStack()
        zallp = zall_est.enter_context(tc.tile_pool(name="zallp", bufs=1))
        zall = zallp.tile([128, NB, E9], F32)
        gate_est = ExitStack()
        gate = gate_est.enter_context(tc.tile_pool(name="gate", bufs=4))
        ps = gate_est.enter_context(tc.tile_pool(name="gps", bufs=2, space="PSUM"))
        for c in range(N // 512):
            zt_ps = ps.tile([E9, 512], F32, tag="zt")
            for k in range(KB):
                xt_t = gate.tile([128, 512], F32, tag="xtt")
                nc.sync.dma_start(
                    xt_t[:], xT.ap()[k * 128:(k + 1) * 128, c * 512:(c + 1) * 512])
                nc.tensor.matmul(zt_ps[:], gw_sb[:, k, :], xt_t[:],
                                 start=(k == 0), stop=(k == KB - 1))
            zt_sb = gate.tile([E9, 512], F32, tag="ztsb")
            nc.scalar.copy(zt_sb[:], zt_ps[:])
            for bb in range(4):
                b = c * 4 + bb
                z_ps = ps.tile([128, E9], F32, tag="zp")
                nc.tensor.transpose(z_ps[:], zt_sb[:, bb * 128:(bb + 1) * 128],
                                    ident[:E9, :E9])
                nc.scalar.copy(zall[:, b, :], z_ps[:])
        gate_est.close()

        # ---- routing: top-2 softmax combine weight for own expert ----
        rt2_est = ExitStack()
        rt2 = rt2_est.enter_context(tc.tile_pool(name="rt2", bufs=1))
        m1 = rt2.tile([128, NB], F32)
        nc.vector.tensor_reduce(m1[:], zall[:], axis=AX.X, op=OP.max)
        eqm = rt2.tile([128, NB, E9], F32)
        nc.vector.tensor_tensor(eqm[:], zall[:],
                                m1[:].to_broadcast([128, NB, E9]), OP.is_equal)
        masked = rt2.tile([128, NB, E9], F32)
        nc.vector.scalar_tensor_tensor(masked[:], in0=eqm[:], scalar=-1e30,
                                       in1=zall[:], op0=OP.mult, op1=OP.add)
        m2 = rt2.tile([128, NB], F32)
        nc.vector.tensor_reduce(m2[:], masked[:], axis=AX.X, op=OP.max)
        d2 = rt2.tile([128, NB], F32)
        nc.vector.tensor_sub(d2[:], m2[:], m1[:])
        em2 = rt2.tile([128, NB], F32)
        nc.scalar.activation(em2[:], d2[:], ACT.Exp)
        den = rt2.tile([128, NB], F32)
        nc.vector.tensor_scalar_add(den[:], em2[:], 1.0)
        rden = rt2.tile([128, NB], F32)
        nc.vector.reciprocal(rden[:], den[:])
        ze = zall[:, :, E]          # own-expert column (dup col 8)
        de = rt2.tile([128, NB], F32)
        nc.vector.tensor_sub(de[:], ze, m1[:])
        eze = rt2.tile([128, NB], F32)
        nc.scalar.activation(eze[:], de[:], ACT.Exp)
        sel = rt2.tile([128, NB], F32)
        nc.vector.tensor_tensor(sel[:], ze, m2[:], OP.is_ge)
        comb = rt2.tile([128, NB], F32)
        nc.vector.tensor_mul(comb[:], eze[:], rden[:])
        nc.vector.tensor_mul(comb[:], comb[:], sel[:])

        # ---- compaction: pos[t] = exclusive prefix count of sel ----
        cps_est = ExitStack()
        ps = cps_est.enter_context(tc.tile_pool(name="cps", bufs=1, space="PSUM"))
        pos_ps = ps.tile([128, NB], F32, tag="pos")
        nc.tensor.matmul(pos_ps[:], u128[:], sel[:], start=True, stop=False)
        tot_ps = ps.tile([1, NB], F32, tag="tot")
        nc.tensor.matmul(tot_ps[:], ones_col[:], sel[:], start=True, stop=True)
        tot_sb = rt2.tile([1, NB], F32)
        nc.scalar.copy(tot_sb[:], tot_ps[:])
        tt_ps = ps.tile([NB, 1], F32, tag="tt")
        nc.tensor.transpose(tt_ps[:], tot_sb[:], ident[:1, :1])
        tt_sb = rt2.tile([NB, 1], F32)
        nc.scalar.copy(tt_sb[:], tt_ps[:])
        cum_ps = ps.tile([NB, 1], F32, tag="cum")
        nc.tensor.matmul(cum_ps[:], u128[:NB, :NB], tt_sb[:], start=True, stop=True)
        cum_sb = rt2.tile([NB, 1], F32)
        nc.scalar.copy(cum_sb[:], cum_ps[:])
        bo_ps = ps.tile([1, NB], F32, tag="bo")
        nc.tensor.transpose(bo_ps[:], cum_sb[:], ident[:NB, :NB])
        bo_sb = rt2.tile([1, NB], F32)
        nc.scalar.copy(bo_sb[:], bo_ps[:])
        nc.tensor.matmul(pos_ps[:], ones_row[:], bo_sb[:], start=False, stop=True)
        pos = rt2.tile([128, NB], F32)
        nc.scalar.copy(pos[:], pos_ps[:])
        cps_est.close()

        offs = rt2.tile([128, NB], F32)
        nc.vector.scalar_tensor_tensor(offs[:], in0=sel[:], scalar=-BIG,
                                       in1=pos[:], op0=OP.mult, op1=OP.add)
        nc.vector.tensor_scalar_add(offs[:], offs[:], BIG)
        offs_i = rt2.tile([128, NB], I32)
        nc.vector.tensor_copy(offs_i[:], offs[:])

        # ---- scatter (t+1, comb) meta pairs into xmeta ----
        metaall = rt2.tile([128, NB, 2], F32)
        nc.vector.tensor_copy(metaall[:, :, 0], iota_sb[:])
        nc.vector.tensor_copy(metaall[:, :, 1], comb[:])
        if batched_meta_scatter:
            nc.gpsimd.indirect_dma_start(
                out=xmeta[:], out_offset=bass.IndirectOffsetOnAxis(
                    ap=offs_i[:, :], axis=0),
                in_=metaall[:], in_offset=None,
                bounds_check=C - 1, oob_is_err=False)
        else:
            for b in range(NB):
                nc.gpsimd.indirect_dma_start(
                    out=xmeta[:], out_offset=bass.IndirectOffsetOnAxis(
                        ap=offs_i[:, b:b + 1], axis=0),
                    in_=metaall[:, b, :], in_offset=None,
                    bounds_check=C - 1, oob_is_err=False)

        # ---- read back compact meta: idx (int16, 16-wrap, replicated) ----
        idxf = rt.tile([128, C // 16], F32)
        for g in range(8):
            nc.sync.dma_start(
                idxf[g * 16:(g + 1) * 16, :],
                xmeta[:, 0:1].rearrange("(cc p) one -> p (cc one)", p=16))
        nc.vector.tensor_scalar_add(idxf[:], idxf[:], -1.0)
        idx16 = rt.tile([128, C // 16], I16)
        nc.vector.tensor_copy(idx16[:], idxf[:])
        idxfg = rt.tile([128, C // 16], F32)
        nc.vector.tensor_scalar_max(idxfg[:], idxf[:], 0.0)
        idx16g = rt.tile([128, C // 16], I16)
        nc.vector.tensor_copy(idx16g[:], idxfg[:])
        comb_g = rt.tile([128, C // 128], F32)
        nc.sync.dma_start(
            comb_g[:], xmeta[:, 1:2].rearrange("(cc p) one -> p (cc one)", p=128))
        rt2_est.close()
        zall_est.close()
        if dbg:
            nc.sync.dma_start(idx_o.ap(), idx16[:])
            nc.sync.dma_start(comb_o.ap(), comb_g[:])

        # ---- FFN over compact chunks (float32r) ----
        ffn_est = ExitStack()
        ffn = ffn_est.enter_context(tc.tile_pool(name="ffn", bufs=1))
        wp = ffn_est.enter_context(tc.tile_pool(name="wpool", bufs=2))
        fps_gu = ffn_est.enter_context(
            tc.tile_pool(name="fps_gu", bufs=2, space="PSUM"))
        fps_y = ffn_est.enter_context(
            tc.tile_pool(name="fps_y", bufs=2, space="PSUM"))
        fps_t = ffn_est.enter_context(
            tc.tile_pool(name="fps_t", bufs=2, space="PSUM"))
        io = ffn_est.enter_context(tc.tile_pool(name="io", bufs=2))

        for ch in range(NCH if stage >= 3 else stage):
            xgch = ffn.tile([128, CB, D], F32, tag="xgch")
            nc.gpsimd.dma_gather(
                xgch[:], x.ap(), idx16g[:, ch * CW:(ch + 1) * CW],
                CHUNK, CHUNK, D)
            if dbg and ch == 0:
                nc.sync.dma_start(xg_o.ap(), xgch[:])
            if stage <= 1:
                continue

            xgT = ffn.tile([128, KB, CHUNK], F32R, tag="xgT_ych")
            for cb in range(CB):
                for k in range(KB):
                    t_ps = fps_t.tile([128, 128], F32, tag="tt")
                    nc.tensor.transpose(
                        t_ps[:], xgch[:, cb, k * 128:(k + 1) * 128], ident[:])
                    nc.scalar.copy(xgT[:, k, cb * 128:(cb + 1) * 128], t_ps[:])

            h = ffn.tile([128, FB, CHUNK], F32R, tag="h")
            for f in range(FB):
                wg_t = wp.tile([128, KB, 128], F32R, tag="wg")
                nc.sync.dma_start(wg_t[:], wgT.ap().rearrange(
                    "(kb p) f -> p kb f", p=128)[:, :, f * 128:(f + 1) * 128])
                wu_t = wp.tile([128, KB, 128], F32R, tag="wu")
                nc.sync.dma_start(wu_t[:], wuT.ap().rearrange(
                    "(kb p) f -> p kb f", p=128)[:, :, f * 128:(f + 1) * 128])
                for n3 in range(N3):
                    nsl = slice(n3 * NMM, (n3 + 1) * NMM)
                    g_ps = fps_gu.tile([128, NMM], F32, tag="g")
                    u_ps = fps_gu.tile([128, NMM], F32, tag="u")
                    for k in range(KB):
                        nc.tensor.matmul(g_ps[:], wg_t[:, k, :], xgT[:, k, nsl],
                                         start=(k == 0), stop=(k == KB - 1))
                    for k in range(KB):
                        nc.tensor.matmul(u_ps[:], wu_t[:, k, :], xgT[:, k, nsl],
                                         start=(k == 0), stop=(k == KB - 1))
                    g_sb = io.tile([128, NMM], F32, tag="gsb")
                    nc.scalar.copy(g_sb[:], g_ps[:])
                    p_sb = io.tile([128, NMM], F32, tag="p")
                    nc.vector.tensor_mul(p_sb[:], g_sb[:], u_ps[:])
                    nc.scalar.activation(h[:, f, nsl], p_sb[:], ACT.Silu)

            y_sb = ffn.tile([128, KB, CHUNK], F32, tag="y")
            for d in range(KB):
                dw_t = wp.tile([128, FB, 128], F32R, tag="dw")
                nc.sync.dma_start(dw_t[:], dwT.ap().rearrange(
                    "(fb p) d -> p fb d", p=128)[:, :, d * 128:(d + 1) * 128])
                for n3 in range(N3):
                    nsl = slice(n3 * NMM, (n3 + 1) * NMM)
                    y_ps = fps_y.tile([128, NMM], F32, tag="yp")
                    for f in range(FB):
                        nc.tensor.matmul(y_ps[:], dw_t[:, f, :], h[:, f, nsl],
                                         start=(f == 0), stop=(f == FB - 1))
                    nc.scalar.copy(y_sb[:, d, nsl], y_ps[:])

            ych = ffn.tile([128, CB, D], F32, tag="xgT_ych")
            for cb in range(CB):
                for d in range(KB):
                    yt_ps = fps_t.tile([128, 128], F32, tag="tt")
                    nc.tensor.transpose(
                        yt_ps[:], y_sb[:, d, cb * 128:(cb + 1) * 128], ident[:])
                    nc.vector.tensor_scalar_mul(
                        ych[:, cb, d * 128:(d + 1) * 128], yt_ps[:],
                        comb_g[:, ch * CB + cb:ch * CB + cb + 1])
            nc.gpsimd.dma_scatter_add(
                partial[:], ych[:], idx16[:, ch * CW:(ch + 1) * CW],
                CHUNK, CHUNK, D)

        ffn_est.close()

        # ---- combine across experts ----
        if with_rs:
            shard = dram.tile([N // n_cores, D], F32)
            nc.gpsimd.collective_compute(
                "ReduceScatter", OP.add,
                replica_groups=[list(range(n_cores))],
                ins=[partial[:].opt()],
                outs=[shard[:].opt()])
            nc.sync.dma_start(shard_o.ap(), shard[:])
        else:
            for r in range(N // 128):
                cp = rt.tile([128, D], F32, tag="cpout")
                nc.sync.dma_start(cp[:], partial[r * 128:(r + 1) * 128, :])
                nc.sync.dma_start(part_o.ap()[r * 128:(r + 1) * 128, :], cp[:])
    nc.compile()
    return nc


def make_core_inputs(x, xT, gate_w, iota, gp_w, up_w, down_w, expert):
    gwT9 = np.ascontiguousarray(
        np.concatenate([gate_w.T, gate_w.T[:, expert:expert + 1]], axis=1))
    pad = FP - F

    def padT(w):  # [F, D] -> [D, FP]
        wt = np.ascontiguousarray(w.T)
        return np.pad(wt, ((0, 0), (0, pad)))

    return {
        "x": x, "xT": xT, "gwT9": gwT9, "iota": iota,
        "wgT": padT(gp_w[expert]),
        "wuT": padT(up_w[expert]),
        "dwT": np.pad(np.ascontiguousarray(down_w[expert].T), ((0, pad), (0, 0))),
    }


_CACHE = {}


def _get_nc():
    if "nc" not in _CACHE:
        nc = bacc.Bacc(trn_type="TRN2", num_devices=NCORES, debug=False)
        build_moe(nc, n_cores=NCORES, with_rs=True)
        _CACHE["nc"] = nc
    return _CACHE["nc"]


def _run(inputs, trace=False):
    x = np.ascontiguousarray(inputs["x"].reshape(N, D).astype(np.float32))
    xT = np.ascontiguousarray(x.T)
    gate_w = inputs["gate_w"].astype(np.float32)
    iota = (np.arange(NB)[None, :] * 128 + np.arange(128)[:, None] + 1
            ).astype(np.float32)
    gp_w = np.asarray(inputs["gp_w"], np.float32)
    up_w = np.asarray(inputs["up_w"], np.float32)
    down_w = np.asarray(inputs["down_w"], np.float32)
    in_maps = [
        make_core_inputs(x, xT, gate_w, iota, gp_w, up_w, down_w, e)
        for e in range(NCORES)
    ]
    nc = _get_nc()
    kw = {"trace_cores": list(range(NCORES))} if trace else {}
    res = run_bass_kernel_spmd(nc, in_maps, core_ids=list(range(NCORES)),
                               trace=trace, **kw)
    shards = [res.results[c]["shard_o"] for c in range(NCORES)]
    y = np.concatenate(shards, axis=0).reshape(B, S, D).astype(np.float32)
    return y, res


def kernel(**inputs):
    y, _ = _run(inputs, trace=False)
    return y

